# revision 72
# baseline (speedup 1.0000x reference)
"""Trainium2 Bass kernel for DeChunking EMA (lower-triangular decay matmul).

Math: out[b,i,:] = sum_{j<=i} exp(S_i - S_j) * p_j * z[b,j,:],
with S = cumsum(log(clip(1-p))). Computed chunked-scan style (Mamba-SSD):

  - L split into C=32 chunks of Q=128, grouped as 4 "quarters" of 8.
  - Intra-chunk: out_intra = W_c^T.T @ z_c with
      W_c^T[j,i] = exp(S'_i - S'_j + log p_j) (masked to i>=j),
    where S' is S re-centered per chunk. The delta matrix is produced on
    the PE as an all-bf16 block-diagonal stacked matmul with the hi/lo
    mantissa split done explicitly on the host (K=4 per chunk:
    1*S'hi_i + 1*S'lo_i + combohi_j*1 + combolo_j*1), which matches the
    fp32r decomposition exactly but runs 1 cycle/col and halves the aux
    DMA. The tril mask is a DVE add of a NEG mask before the ACT exp.
  - Inter-chunk: chunk states H_c = U_c^T @ z_c accumulate per quarter
    into an [8,192] PSUM block via a block-diagonal pre-exp'd U; a
    quarter-blocked decay matmul (M2) forms carry rows; kappa*carry is
    folded into row 0 of each z quarter (gpsimd accum DMA), so the intra
    matmul's W^T row 0 applies the rank-1 carry term for free.

DMA strategy (the dominant constraint): every DMA instruction costs
~0.6-1.0us of queue dead time and per-queue arbitration starves any
queue behind the busiest one, so ALL inputs ride the sync HWDGE queue
as SIX transfers in need order: packed delta stacks ([16, 5120] bf16,
16 partitions -> all 16 DMA engines), then [awu | z quarter 0] as one
transfer (awu = pre-exp'd U + NEG mask + M2 packed into z's row head),
then z quarters 1-3. Out stores are 3 quarter-slabs + 2 half-slabs on
the same queue. Fold accum-DMAs are gpsimd SWDGE (the only accum path).

Schedule: 4 junk warmup matmuls, then per-quarter software pipeline
emitted in true data-ready order (H -> castH -> delta -> carry ->
castC -> fold -> delta -> out one quarter behind its fold), so the
static per-engine FIFOs have no head-of-line blocking. PSUM: 3 delta
bufs + 3 out-pair bufs + 1 H/carry tile = 8 banks.

Sharding (8 cores, no collectives): core = (batch b in {0,1}) x (one of
4 D-blocks of 192). Each core reads z[b, :, blk] and pt[b] only.
"""

import os
import numpy as np
import ml_dtypes

B, L, D = 2, 4096, 768
Q = 128
C = L // Q           # 32 chunks
ND = 4               # D blocks per batch
DBLK = D // ND       # 192
GRP = 4              # chunks per delta/exp group
NG = C // GRP        # 8 groups
NEG = -3.0e38
N_CORES = 8
NZQ = 4              # z quarter tiles (chunks 8q..8q+7)
ZCH = C // NZQ       # 8 chunks per quarter
NODMA = 4            # out-store DMA splits (aligned to z quarters)

_CTX = {}
LAST_EXEC_NS = None


def _build_program():
    import concourse.bacc as bacc
    import concourse.mybir as mybir
    from concourse import tile

    f32 = mybir.dt.float32
    f32r = mybir.dt.float32r
    bf16 = mybir.dt.bfloat16
    nc = bacc.Bacc("TRN2", target_bir_lowering=False, debug=False,
                   num_devices=N_CORES, num_swdge_queues=4)

    FD = C * DBLK  # 6144 free elems in the big position-major tiles
    QW = ZCH * DBLK  # 1536 free elems per z quarter
    A12W = NG * Q + NG * GRP * Q  # 5120: K=2/chunk delta stack width
    AXW = A12W
    AWUW = C * ZCH + GRP * Q + NZQ * C  # Ublk | mask | M2 (rows 0-7)
    # awu is prepended to z ([awu | z] per row) so one DMA delivers both
    # awu and z quarter 0 -> the H/W prep chains unlock with zq0.
    z_s = nc.dram_tensor("z_s", [Q, AWUW + FD], bf16, kind="ExternalInput")
    # delta stacks as explicit hi/lo bf16 (K=4 per chunk): same exact
    # products as the fp32r path (operands are pre-rounded to bf16 hi+lo)
    # but 1 cycle/col on the PE instead of 2, half the DMA bytes, and the
    # 16-partition layout engages all 16 DMA engines
    aux_all = nc.dram_tensor("aux_all", [4 * GRP, AXW], bf16,
                             kind="ExternalInput")
    out_s = nc.dram_tensor("out_s", [Q, FD], bf16, kind="ExternalOutput")

    Exp = mybir.ActivationFunctionType.Exp

    with tile.TileContext(nc) as tc:
        with (
            tc.tile_pool(name="zp", bufs=1) as zp,
            tc.tile_pool(name="wp", bufs=NG) as wp,
            tc.tile_pool(name="sp", bufs=1) as sp,
            tc.tile_pool(name="dps", bufs=4, space="PSUM") as dps,
            tc.tile_pool(name="ops", bufs=3, space="PSUM") as ops,
            tc.tile_pool(name="hps", bufs=1, space="PSUM") as hps,
        ):
            # Single-queue input stream: HWDGE queue arbitration starves
            # whichever queue isn't first, so ALL inputs ride the sync
            # queue in exact need order (packed aux for the W/H prep
            # chains first, then the z quarters).
            # azt holds [awu | z quarter 0]; filled by one DMA
            azt = zp.tile([Q, AWUW + QW], bf16, tag="azt")
            zq = [azt]  # zq[0] accessed via AWUW offset below
            for s in range(1, NZQ):
                t = zp.tile([Q, QW], bf16, tag=f"z{s}")
                zq.append(t)
            auxt = sp.tile([4 * GRP, AXW], bf16, tag="auxt")
            nc.sync.dma_start(auxt[:], aux_all[:])
            nc.sync.dma_start(azt[:], z_s[:, 0:AWUW + QW])
            for s in range(1, NZQ):
                nc.sync.dma_start(
                    zq[s][:], z_s[:, AWUW + s * QW:AWUW + (s + 1) * QW])

            a12 = auxt  # delta stacks live in cols [0, A12W)
            Ublk = azt  # U slabs in cols [0, C*ZCH); mask after
            M2B = C * ZCH + GRP * Q  # M2 block base (rows 0-7 used)

            def m2slab(q, q2):
                # [8, 8] bf16 view of the (q2 -> q) decay block
                base = M2B + q * C + q2 * ZCH
                return azt[0:ZCH, base:base + ZCH]

            aw = azt[:, C * ZCH:M2B]

            def zrow0(s):
                # row-0 slice of z quarter s (fold DMA destination)
                if s == 0:
                    return azt[0:1, AWUW:AWUW + QW]
                return zq[s][0:1, :]

            wm_sb = sp.tile([Q, 2 * Q], bf16, tag="wm_sb")
            nc.gpsimd.memset(wm_sb[:], 1.0)

            def zchunk(c):
                s, r = divmod(c, ZCH)
                if s == 0:
                    return azt[:, AWUW + r * DBLK:AWUW + (r + 1) * DBLK]
                return zq[s][:, r * DBLK:(r + 1) * DBLK]

            sL = a12[:, 0:NG * Q]
            sR = a12[:, NG * Q:A12W]

            # PE clock warmup: junk matmuls bridge until z/aux land and
            # start filling the HAM SHORT window so the PE reaches K=8/8
            # (2.4 GHz) shortly after real work begins.
            wm_ps = ops.tile([Q, 2, 256], f32, tag="o")

            def junk(n, width=256):
                # head warmup: free-floating junk matmuls (the scheduler
                # hoists them to the front, which is where we want them)
                for _ in range(n):
                    nc.tensor.matmul(wm_ps[:, 0, 0:width],
                                     wm_sb[:, 0:Q], wm_sb[:, 0:width])

            junk(4)

            wT = []

            def delta_group(g):
                dp = dps.tile([Q, GRP * Q], f32, tag="dp")
                nc.tensor.matmul(
                    dp[:],
                    sL[:, g * Q:(g + 1) * Q],
                    sR[:, g * GRP * Q:(g + 1) * GRP * Q],
                    start=True, stop=True,
                )
                # tril mask: DVE adds the tiled strict-upper NEG mask onto
                # the delta PSUM before the exp (NEG dominates any finite
                # delta, so exp gives exact zeros above the diagonal)
                nc.vector.tensor_add(dp[:], dp[:], aw[:])
                w4 = wp.tile([Q, GRP * Q], bf16, tag="w4")
                nc.scalar.activation(w4[:], dp[:], Exp)
                wT.append(w4)

            # Per-quarter software pipeline. H/carry/fold for quarter q run
            # as soon as z quarter q lands; out pairs lag one quarter so
            # the fold DMA latency hides under the next quarter's H work.
            # one-bank H/carry tile: quarters alternate 2 slots (quarter
            # q+2's H matmuls start ~2 quarters after castC_q drains its
            # slot, so the WAR dependency is off the critical path)
            hc_ps = hps.tile([ZCH, 2, 256], f32, tag="h")

            def hblk(q):
                return hc_ps[:, q % 2, 0:DBLK]

            H = sp.tile([ZCH, NZQ * DBLK], bf16, tag="H")
            cfk = sp.tile([ZCH, NZQ * DBLK], bf16, tag="cfk")
            osb = sp.tile([Q, FD], bf16, tag="osb")
            ssl = FD // NODMA

            def h_part(q):
                # H state matmuls for the 8 chunks of quarter q: chunk c
                # writes row (c%8) of one [8,192] PSUM accumulation block
                for r in range(ZCH):
                    c = q * ZCH + r
                    nc.tensor.matmul(
                        hblk(q),
                        Ublk[:, c * ZCH:(c + 1) * ZCH],
                        zchunk(c),
                        start=(r == 0), stop=(r == ZCH - 1),
                        skip_group_check=True,
                    )
                qsl = slice(q * DBLK, (q + 1) * DBLK)
                nc.vector.tensor_copy(H[:, qsl], hblk(q))

            def carry_fold(q):
                qsl = slice(q * DBLK, (q + 1) * DBLK)
                # carry for quarter q sums decayed H states of quarters
                # q2 <= q (kappa*carry overwrites PSUM block q)
                for q2 in range(q + 1):
                    nc.tensor.matmul(
                        hblk(q),
                        m2slab(q, q2),
                        H[:, q2 * DBLK:(q2 + 1) * DBLK],
                        start=(q2 == 0), stop=(q2 == q),
                        skip_group_check=True,
                    )
                nc.vector.tensor_copy(cfk[:, qsl], hblk(q))
                # fold kappa*carry into row 0 of z quarter q (out += a (x)
                # carry == W^T row 0 applying the rank-1 update once
                # z[0] += kappa*carry)
                nc.gpsimd.dma_start(
                    zrow0(q),
                    cfk[:, qsl],
                    accum_op=mybir.AluOpType.add,
                )

            def out_pair(p):
                # 2 chunk matmuls into one [128, 2, 256] PSUM tile (each
                # chunk 256-aligned inside one bank), one strided cast
                # (alternating DVE/ACT); store 196KB after every 2nd pair
                o_ps = ops.tile([Q, 2, 256], f32, tag="o")
                for h in range(2):
                    c = 2 * p + h
                    g, k = divmod(c, GRP)
                    nc.tensor.matmul(
                        o_ps[:, h, 0:DBLK],
                        wT[g][:, k * Q:(k + 1) * Q],
                        zchunk(c),
                    )
                osl = slice(2 * p * DBLK, (2 * p + 2) * DBLK)
                if p % 2 == 0:
                    nc.vector.tensor_copy(osb[:, osl], o_ps[:, :, 0:DBLK])
                else:
                    nc.scalar.copy(osb[:, osl], o_ps[:, :, 0:DBLK])
                # store quarter slabs (plus two half-slabs at the tail so
                # the final store is small): each DMA has ~0.7us of queue
                # overhead, so fewer/bigger stores win
                stores = {3: (0, 8), 7: (8, 16), 11: (16, 24),
                          13: (24, 28), 15: (28, 32)}
                if p in stores:
                    lo, hi = stores[p]
                    ssl2 = slice(lo * DBLK, hi * DBLK)
                    nc.sync.dma_start(out_s[:, ssl2], osb[:, ssl2])

            def out_quarter(oq):
                for p in range(4 * oq, 4 * oq + 4):
                    out_pair(p)

            # Pipeline, emitted in true data-ready order so the static
            # per-engine schedules have no head-of-line blocking:
            # H/carry/fold(q) as z quarter q lands; two delta groups
            # after each fold keep the PE fed; out quarter q runs one
            # quarter behind its fold so the fold DMA latency stays off
            # the critical path.
            delta_group(0)
            delta_group(1)
            h_part(0)
            delta_group(2)
            carry_fold(0)
            delta_group(3)
            for q in range(1, NZQ):
                h_part(q)
                if q < NZQ - 1:
                    delta_group(2 * q + 2)
                carry_fold(q)
                if q < NZQ - 1:
                    delta_group(2 * q + 3)
                out_quarter(q - 1)
            out_quarter(NZQ - 1)

    nc.compile()
    return nc


def _host_prep(pt_b):
    """Per-batch host-side prep of the small scan operands. pt_b: [L] f32."""
    pt_b = pt_b.astype(np.float64)
    decay = np.clip(1.0 - pt_b, 1e-12, None)
    S = np.cumsum(np.log(decay))
    logp = np.log(np.maximum(pt_b, 1e-38))
    Send = S[Q - 1::Q]
    Sendprev = np.concatenate([[0.0], Send[:-1]])

    Sm = S.reshape(C, Q)
    logpm = logp.reshape(C, Q)
    # Re-center S within each chunk (see module docstring) and pre-round
    # operands to bf16-hi+lo representable values so the fp32r matmul
    # decomposition is exact.
    Sc = Sm - Sm[:, :1]

    def r16hl(x):
        h = x.astype(ml_dtypes.bfloat16).astype(np.float64)
        l = (x - h).astype(ml_dtypes.bfloat16).astype(np.float64)
        return h, l

    ScRaw = Sc
    Schi, Sclo = r16hl(Sc)
    Sc = Schi + Sclo
    # combined j-operand rows: delta[j,i] = Sc_i + (logp - Sc)_j, split
    # into explicit bf16 hi/lo rank-1 terms (K=4/chunk, all-bf16 PE path)
    chi, clo = r16hl(logpm - ScRaw)
    combo = chi + clo

    stackL = np.zeros((4 * GRP, NG * Q), np.float64)
    stackR = np.zeros((4 * GRP, NG * GRP * Q), np.float64)
    for g in range(NG):
        for k in range(GRP):
            c = g * GRP + k
            lcol = slice(g * Q, (g + 1) * Q)
            stackL[4 * k + 0, lcol] = 1.0
            stackL[4 * k + 1, lcol] = 1.0
            stackL[4 * k + 2, lcol] = chi[c]
            stackL[4 * k + 3, lcol] = clo[c]
            rcol = slice(g * GRP * Q + k * Q, g * GRP * Q + (k + 1) * Q)
            stackR[4 * k + 0, rcol] = Schi[c]
            stackR[4 * k + 1, rcol] = Sclo[c]
            stackR[4 * k + 2, rcol] = 1.0
            stackR[4 * k + 3, rcol] = 1.0

    # U block-diagonal, pre-exp'd bf16: chunk c's u vector at column
    # c*8 + (c%8) of a zero [Q, C*8] tile (so the [128,8] slab for chunk
    # c writes row c%8 of its quarter's H block)
    uexp = (Send[:, None] - Sm + logpm).T  # [Q, C]
    ublk = np.zeros((Q, C * ZCH), np.float64)
    for c in range(C):
        ublk[:, c * ZCH + (c % ZCH)] = np.exp(uexp[:, c])
    ublk = ublk.astype(ml_dtypes.bfloat16)

    # log kappa_c = S_{c,0} - Send_{c-1} - combo[c,0]: scaling such that
    # W^T row 0 (= exp(S'_i + combo[c,0])) times kappa*carry reproduces the
    # rank-1 carry term a_i*carry. Uses the device-rounded combo so the
    # coefficient reconstruction cancels exactly. Folded into the decay
    # matrix exponents host-side (dest column c).
    logkap = np.minimum(Sm[:, 0] - Sendprev - combo[:, 0], 69.0)

    # quarter-blocked decay exponents: block (q2, q) at cols q*C + q2*ZCH,
    # rows = source chunk within q2, cols-in-block = dest chunk within q
    d2qb = np.full((ZCH, NZQ * C), NEG, np.float64)
    for q in range(NZQ):
        for q2 in range(q + 1):
            for cr in range(ZCH):          # source chunk c2 = q2*8 + cr
                for cc in range(ZCH):      # dest chunk c = q*8 + cc
                    c2 = q2 * ZCH + cr
                    c = q * ZCH + cc
                    if c2 < c:
                        d2qb[cr, q * C + q2 * ZCH + cc] = (
                            Sendprev[c] - Send[c2] + logkap[c]
                        )
    m2 = np.exp(d2qb).astype(ml_dtypes.bfloat16)

    aux_all = np.concatenate([stackL, stackR],
                             axis=1).astype(ml_dtypes.bfloat16)

    # pack: awu = [Ublk bf16 | NEG tril mask bf16 | M2 on rows 0-7]
    m2pad = np.zeros((Q, NZQ * C), ml_dtypes.bfloat16)
    m2pad[0:ZCH] = m2
    awu = np.concatenate([ublk, _get_auxw(), m2pad], axis=1)
    return aux_all, awu


_AUXW = None


def _get_auxw():
    """bf16 [Q, GRP*Q]: tiled strict-upper NEG mask."""
    global _AUXW
    if _AUXW is None:
        j = np.arange(Q)[:, None]
        i = np.arange(Q)[None, :]
        one = np.where(i >= j, 0.0, NEG)
        _AUXW = np.tile(one, (1, GRP)).astype(ml_dtypes.bfloat16)
    return _AUXW


def _make_in_maps(z, pt):
    preps = [_host_prep(pt[b]) for b in range(B)]
    in_maps = []
    for core in range(N_CORES):
        b, dblk = divmod(core, ND)
        aux_all, awu = preps[b]
        z_slab = (
            z[b, :, dblk * DBLK:(dblk + 1) * DBLK]
            .reshape(C, Q, DBLK)
            .transpose(1, 0, 2)
            .reshape(Q, C * DBLK)
            .astype(ml_dtypes.bfloat16)
        )
        in_maps.append({
            # [awu | z] per row; one DMA delivers awu + z quarter 0
            "z_s": np.ascontiguousarray(
                np.concatenate([awu, z_slab], axis=1)),
            "aux_all": np.ascontiguousarray(aux_all),
        })
    return in_maps


def _unpack_out(res_core):
    """out_s [Q, C*DBLK] bf16 position-major -> [L, DBLK] f32."""
    return (
        res_core.astype(np.float32)
        .reshape(Q, C, DBLK)
        .transpose(1, 0, 2)
        .reshape(L, DBLK)
    )


def _install_ntff_shim():
    """Enable NTFF profiling under axon: shim the missing antenv.axon_hooks
    module and register the ctypes hook from trn_boot; skip the fileshare
    artifact upload (no bucket in this container)."""
    import sys
    import types
    import antenv

    if "antenv.axon_hooks" not in sys.modules:
        mod = types.ModuleType("antenv.axon_hooks")
        hook_box = [None]
        mod.set_axon_ntff_profile_hook = lambda h: hook_box.__setitem__(0, h)
        mod.get_axon_ntff_profile_hook = lambda: hook_box[0]
        mod._hook_box = hook_box
        sys.modules["antenv.axon_hooks"] = mod
        antenv.axon_hooks = mod
    mod = sys.modules["antenv.axon_hooks"]
    if mod.get_axon_ntff_profile_hook() is None:
        from trn_agent_boot.trn_boot import _ntff_profile_via_ctypes

        mod.set_axon_ntff_profile_hook(
            _ntff_profile_via_ctypes("/opt/axon/libaxon_pjrt.so")
        )
    import concourse.bass_utils as bu

    bu.upload_artifacts = lambda tmpdir: f"local://{tmpdir}"


def kernel(z, pt):
    global LAST_EXEC_NS
    from concourse.bass_utils import run_bass_kernel_spmd

    z = np.asarray(z, dtype=np.float32)
    pt = np.asarray(pt, dtype=np.float32)

    if "nc" not in _CTX:
        _CTX["nc"] = _build_program()
    nc = _CTX["nc"]

    in_maps = _make_in_maps(z, pt)

    trace = bool(int(os.environ.get("BASS_KERNEL_TRACE", "0")))
    if trace:
        try:
            _install_ntff_shim()
        except Exception:
            trace = False
    tmpdir = os.environ.get("BASS_KERNEL_TRACE_DIR") or None
    res = run_bass_kernel_spmd(
        nc, in_maps, list(range(N_CORES)), trace=trace, tmpdir=tmpdir
    )
    LAST_EXEC_NS = res.exec_time_ns

    out = np.empty((B, L, D), np.float32)
    for core in range(N_CORES):
        b, dblk = divmod(core, ND)
        out[b, :, dblk * DBLK:(dblk + 1) * DBLK] = _unpack_out(
            res.results[core]["out_s"]
        )
    return out



# revision 73
# speedup vs baseline: 1.1132x; 1.1132x over previous
"""Trainium2 Bass kernel for DeChunking EMA (lower-triangular decay matmul).

Math: out[b,i,:] = sum_{j<=i} exp(S_i - S_j) * p_j * z[b,j,:],
with S = cumsum(log(clip(1-p))). Computed chunked-scan style (Mamba-SSD):

  - L split into C=32 chunks of Q=128, grouped as 4 "quarters" of 8.
  - Intra-chunk: out_intra = W_c^T.T @ z_c with
      W_c^T[j,i] = exp(S'_i - S'_j + log p_j) (masked to i>=j),
    where S' is S re-centered per chunk. The delta matrix is produced on
    the PE as an all-bf16 block-diagonal stacked matmul with the hi/lo
    mantissa split done explicitly on the host (K=4 per chunk:
    1*S'hi_i + 1*S'lo_i + combohi_j*1 + combolo_j*1), which matches the
    fp32r decomposition exactly but runs 1 cycle/col and halves the aux
    DMA. The tril mask is a DVE add of a NEG mask before the ACT exp.
  - Inter-chunk: chunk states H_c = U_c^T @ z_c accumulate per quarter
    into an [8,192] PSUM block via a block-diagonal pre-exp'd U; a
    quarter-blocked decay matmul (M2) forms carry rows; kappa*carry is
    folded into row 0 of each z quarter (gpsimd accum DMA), so the intra
    matmul's W^T row 0 applies the rank-1 carry term for free.

DMA strategy (the dominant constraint): every DMA instruction costs
~0.6-1.0us of queue dead time and per-queue arbitration starves any
queue behind the busiest one, so ALL inputs ride the sync HWDGE queue
as SIX transfers in need order: packed delta stacks ([16, 5120] bf16,
16 partitions -> all 16 DMA engines), then [awu | z quarter 0] as one
transfer (awu = pre-exp'd U + NEG mask + M2 packed into z's row head),
then z quarters 1-3. Out stores are 3 quarter-slabs + 2 half-slabs on
the same queue. Fold accum-DMAs are gpsimd SWDGE (the only accum path).

Schedule: 4 junk warmup matmuls, then per-quarter software pipeline
emitted in true data-ready order (H -> castH -> delta -> carry ->
castC -> fold -> delta -> out one quarter behind its fold), so the
static per-engine FIFOs have no head-of-line blocking. PSUM: 3 delta
bufs + 3 out-pair bufs + 1 H/carry tile = 8 banks.

Sharding (8 cores, no collectives): core = (batch b in {0,1}) x (one of
4 D-blocks of 192). Each core reads z[b, :, blk] and pt[b] only.
"""

import os
import numpy as np
import ml_dtypes

B, L, D = 2, 4096, 768
Q = 128
C = L // Q           # 32 chunks
ND = 4               # D blocks per batch
DBLK = D // ND       # 192
GRP = 4              # chunks per delta/exp group
NG = C // GRP        # 8 groups
NEG = -3.0e38
N_CORES = 8
NZQ = 4              # z quarter tiles (chunks 8q..8q+7)
ZCH = C // NZQ       # 8 chunks per quarter
NODMA = 4            # out-store DMA splits (aligned to z quarters)

_CTX = {}
LAST_EXEC_NS = None


def _build_program():
    import concourse.bacc as bacc
    import concourse.mybir as mybir
    from concourse import tile

    f32 = mybir.dt.float32
    f32r = mybir.dt.float32r
    bf16 = mybir.dt.bfloat16
    nc = bacc.Bacc("TRN2", target_bir_lowering=False, debug=False,
                   num_devices=N_CORES, num_swdge_queues=4)

    FD = C * DBLK  # 6144 free elems in the big position-major tiles
    QW = ZCH * DBLK  # 1536 free elems per z quarter
    A12W = NG * Q + NG * GRP * Q  # 5120: K=2/chunk delta stack width
    AXW = A12W
    AWUW = C * ZCH + GRP * Q + NZQ * C  # Ublk | mask | M2 (rows 0-7)
    # awu is prepended to z ([awu | z] per row) so one DMA delivers both
    # awu and z quarter 0 -> the H/W prep chains unlock with zq0.
    z_s = nc.dram_tensor("z_s", [Q, AWUW + FD], bf16, kind="ExternalInput")
    # delta stacks as explicit hi/lo bf16 (K=4 per chunk): same exact
    # products as the fp32r path (operands are pre-rounded to bf16 hi+lo)
    # but 1 cycle/col on the PE instead of 2, half the DMA bytes, and the
    # 16-partition layout engages all 16 DMA engines
    aux_all = nc.dram_tensor("aux_all", [4 * GRP, AXW], bf16,
                             kind="ExternalInput")
    out_s = nc.dram_tensor("out_s", [Q, FD], bf16, kind="ExternalOutput")

    Exp = mybir.ActivationFunctionType.Exp

    with tile.TileContext(nc) as tc:
        with (
            tc.tile_pool(name="zp", bufs=1) as zp,
            tc.tile_pool(name="wp", bufs=NG) as wp,
            tc.tile_pool(name="sp", bufs=1) as sp,
            tc.tile_pool(name="dps", bufs=3, space="PSUM") as dps,
            tc.tile_pool(name="ops", bufs=4, space="PSUM") as ops,
            tc.tile_pool(name="hps", bufs=1, space="PSUM") as hps,
        ):
            # Single-queue input stream: HWDGE queue arbitration starves
            # whichever queue isn't first, so ALL inputs ride the sync
            # queue in exact need order (packed aux for the W/H prep
            # chains first, then the z quarters).
            # azt holds [awu | z quarter 0]; filled by one DMA
            azt = zp.tile([Q, AWUW + QW], bf16, tag="azt")
            zq = [azt]  # zq[0] accessed via AWUW offset below
            for s in range(1, NZQ):
                t = zp.tile([Q, QW], bf16, tag=f"z{s}")
                zq.append(t)
            auxt = sp.tile([4 * GRP, AXW], bf16, tag="auxt")
            nc.sync.dma_start(auxt[:], aux_all[:])
            nc.sync.dma_start(azt[:], z_s[:, 0:AWUW + QW])
            for s in range(1, NZQ):
                nc.sync.dma_start(
                    zq[s][:], z_s[:, AWUW + s * QW:AWUW + (s + 1) * QW])

            a12 = auxt  # delta stacks live in cols [0, A12W)
            Ublk = azt  # U slabs in cols [0, C*ZCH); mask after
            M2B = C * ZCH + GRP * Q  # M2 block base (rows 0-7 used)

            def m2slab(q, q2):
                # [8, 8] bf16 view of the (q2 -> q) decay block
                base = M2B + q * C + q2 * ZCH
                return azt[0:ZCH, base:base + ZCH]

            aw = azt[:, C * ZCH:M2B]

            def zrow0(s):
                # row-0 slice of z quarter s (fold DMA destination)
                if s == 0:
                    return azt[0:1, AWUW:AWUW + QW]
                return zq[s][0:1, :]

            wm_sb = sp.tile([Q, 2 * Q], bf16, tag="wm_sb")
            nc.gpsimd.memset(wm_sb[:], 1.0)

            def zchunk(c):
                s, r = divmod(c, ZCH)
                if s == 0:
                    return azt[:, AWUW + r * DBLK:AWUW + (r + 1) * DBLK]
                return zq[s][:, r * DBLK:(r + 1) * DBLK]

            sL = a12[:, 0:NG * Q]
            sR = a12[:, NG * Q:A12W]

            # PE clock warmup: junk matmuls bridge until z/aux land and
            # start filling the HAM SHORT window so the PE reaches K=8/8
            # (2.4 GHz) shortly after real work begins.
            wm_ps = ops.tile([Q, 2, 256], f32, tag="o")

            def junk(n, width=256):
                # head warmup: free-floating junk matmuls (the scheduler
                # hoists them to the front, which is where we want them)
                for _ in range(n):
                    nc.tensor.matmul(wm_ps[:, 0, 0:width],
                                     wm_sb[:, 0:Q], wm_sb[:, 0:width])

            junk(4)

            wT = []

            def delta_group(g):
                dp = dps.tile([Q, GRP * Q], f32, tag="dp")
                nc.tensor.matmul(
                    dp[:],
                    sL[:, g * Q:(g + 1) * Q],
                    sR[:, g * GRP * Q:(g + 1) * GRP * Q],
                    start=True, stop=True,
                )
                # tril mask: DVE adds the tiled strict-upper NEG mask onto
                # the delta PSUM before the exp (NEG dominates any finite
                # delta, so exp gives exact zeros above the diagonal)
                nc.vector.tensor_add(dp[:], dp[:], aw[:])
                w4 = wp.tile([Q, GRP * Q], bf16, tag="w4")
                nc.scalar.activation(w4[:], dp[:], Exp)
                wT.append(w4)

            # Per-quarter software pipeline. H/carry/fold for quarter q run
            # as soon as z quarter q lands; out pairs lag one quarter so
            # the fold DMA latency hides under the next quarter's H work.
            # one-bank H/carry tile: quarters alternate 2 slots (quarter
            # q+2's H matmuls start ~2 quarters after castC_q drains its
            # slot, so the WAR dependency is off the critical path)
            hc_ps = hps.tile([ZCH, 2, 256], f32, tag="h")

            def hblk(q):
                return hc_ps[:, q % 2, 0:DBLK]

            H = sp.tile([ZCH, NZQ * DBLK], bf16, tag="H")
            cfk = sp.tile([ZCH, NZQ * DBLK], bf16, tag="cfk")
            osb = sp.tile([Q, FD], bf16, tag="osb")
            ssl = FD // NODMA

            def h_part(q):
                # H state matmuls for the 8 chunks of quarter q: chunk c
                # writes row (c%8) of one [8,192] PSUM accumulation block
                for r in range(ZCH):
                    c = q * ZCH + r
                    nc.tensor.matmul(
                        hblk(q),
                        Ublk[:, c * ZCH:(c + 1) * ZCH],
                        zchunk(c),
                        start=(r == 0), stop=(r == ZCH - 1),
                        skip_group_check=True,
                    )
                qsl = slice(q * DBLK, (q + 1) * DBLK)
                nc.vector.tensor_copy(H[:, qsl], hblk(q))

            def carry_fold(q):
                qsl = slice(q * DBLK, (q + 1) * DBLK)
                # carry for quarter q sums decayed H states of quarters
                # q2 <= q (kappa*carry overwrites PSUM block q)
                for q2 in range(q + 1):
                    nc.tensor.matmul(
                        hblk(q),
                        m2slab(q, q2),
                        H[:, q2 * DBLK:(q2 + 1) * DBLK],
                        start=(q2 == 0), stop=(q2 == q),
                        skip_group_check=True,
                    )
                nc.vector.tensor_copy(cfk[:, qsl], hblk(q))
                # fold kappa*carry into row 0 of z quarter q (out += a (x)
                # carry == W^T row 0 applying the rank-1 update once
                # z[0] += kappa*carry)
                nc.gpsimd.dma_start(
                    zrow0(q),
                    cfk[:, qsl],
                    accum_op=mybir.AluOpType.add,
                )

            def out_pair(p):
                # 2 chunk matmuls into one [128, 2, 256] PSUM tile (each
                # chunk 256-aligned inside one bank), one strided cast
                # (alternating DVE/ACT); store 196KB after every 2nd pair
                o_ps = ops.tile([Q, 2, 256], f32, tag="o")
                for h in range(2):
                    c = 2 * p + h
                    g, k = divmod(c, GRP)
                    nc.tensor.matmul(
                        o_ps[:, h, 0:DBLK],
                        wT[g][:, k * Q:(k + 1) * Q],
                        zchunk(c),
                    )
                osl = slice(2 * p * DBLK, (2 * p + 2) * DBLK)
                if p % 2 == 0:
                    nc.vector.tensor_copy(osb[:, osl], o_ps[:, :, 0:DBLK])
                else:
                    nc.scalar.copy(osb[:, osl], o_ps[:, :, 0:DBLK])
                # store quarter slabs (plus two half-slabs at the tail so
                # the final store is small): each DMA has ~0.7us of queue
                # overhead, so fewer/bigger stores win
                stores = {3: (0, 8), 7: (8, 16), 11: (16, 24),
                          13: (24, 28), 15: (28, 32)}
                if p in stores:
                    lo, hi = stores[p]
                    ssl2 = slice(lo * DBLK, hi * DBLK)
                    nc.sync.dma_start(out_s[:, ssl2], osb[:, ssl2])

            def out_quarter(oq):
                for p in range(4 * oq, 4 * oq + 4):
                    out_pair(p)

            # Pipeline, emitted in true data-ready order so the static
            # per-engine schedules have no head-of-line blocking:
            # H/carry/fold(q) as z quarter q lands; two delta groups
            # after each fold keep the PE fed; out quarter q runs one
            # quarter behind its fold so the fold DMA latency stays off
            # the critical path.
            delta_group(0)
            delta_group(1)
            h_part(0)
            carry_fold(0)
            delta_group(2)
            delta_group(3)
            for q in range(1, NZQ):
                h_part(q)
                carry_fold(q)
                if q < NZQ - 1:
                    delta_group(2 * q + 2)
                    delta_group(2 * q + 3)
                out_quarter(q - 1)
            out_quarter(NZQ - 1)

    nc.compile()
    return nc


def _host_prep(pt_b):
    """Per-batch host-side prep of the small scan operands. pt_b: [L] f32."""
    pt_b = pt_b.astype(np.float64)
    decay = np.clip(1.0 - pt_b, 1e-12, None)
    S = np.cumsum(np.log(decay))
    logp = np.log(np.maximum(pt_b, 1e-38))
    Send = S[Q - 1::Q]
    Sendprev = np.concatenate([[0.0], Send[:-1]])

    Sm = S.reshape(C, Q)
    logpm = logp.reshape(C, Q)
    # Re-center S within each chunk (see module docstring) and pre-round
    # operands to bf16-hi+lo representable values so the fp32r matmul
    # decomposition is exact.
    Sc = Sm - Sm[:, :1]

    def r16hl(x):
        h = x.astype(ml_dtypes.bfloat16).astype(np.float64)
        l = (x - h).astype(ml_dtypes.bfloat16).astype(np.float64)
        return h, l

    ScRaw = Sc
    Schi, Sclo = r16hl(Sc)
    Sc = Schi + Sclo
    # combined j-operand rows: delta[j,i] = Sc_i + (logp - Sc)_j, split
    # into explicit bf16 hi/lo rank-1 terms (K=4/chunk, all-bf16 PE path)
    chi, clo = r16hl(logpm - ScRaw)
    combo = chi + clo

    stackL = np.zeros((4 * GRP, NG * Q), np.float64)
    stackR = np.zeros((4 * GRP, NG * GRP * Q), np.float64)
    for g in range(NG):
        for k in range(GRP):
            c = g * GRP + k
            lcol = slice(g * Q, (g + 1) * Q)
            stackL[4 * k + 0, lcol] = 1.0
            stackL[4 * k + 1, lcol] = 1.0
            stackL[4 * k + 2, lcol] = chi[c]
            stackL[4 * k + 3, lcol] = clo[c]
            rcol = slice(g * GRP * Q + k * Q, g * GRP * Q + (k + 1) * Q)
            stackR[4 * k + 0, rcol] = Schi[c]
            stackR[4 * k + 1, rcol] = Sclo[c]
            stackR[4 * k + 2, rcol] = 1.0
            stackR[4 * k + 3, rcol] = 1.0

    # U block-diagonal, pre-exp'd bf16: chunk c's u vector at column
    # c*8 + (c%8) of a zero [Q, C*8] tile (so the [128,8] slab for chunk
    # c writes row c%8 of its quarter's H block)
    uexp = (Send[:, None] - Sm + logpm).T  # [Q, C]
    ublk = np.zeros((Q, C * ZCH), np.float64)
    for c in range(C):
        ublk[:, c * ZCH + (c % ZCH)] = np.exp(uexp[:, c])
    ublk = ublk.astype(ml_dtypes.bfloat16)

    # log kappa_c = S_{c,0} - Send_{c-1} - combo[c,0]: scaling such that
    # W^T row 0 (= exp(S'_i + combo[c,0])) times kappa*carry reproduces the
    # rank-1 carry term a_i*carry. Uses the device-rounded combo so the
    # coefficient reconstruction cancels exactly. Folded into the decay
    # matrix exponents host-side (dest column c).
    logkap = np.minimum(Sm[:, 0] - Sendprev - combo[:, 0], 69.0)

    # quarter-blocked decay exponents: block (q2, q) at cols q*C + q2*ZCH,
    # rows = source chunk within q2, cols-in-block = dest chunk within q
    d2qb = np.full((ZCH, NZQ * C), NEG, np.float64)
    for q in range(NZQ):
        for q2 in range(q + 1):
            for cr in range(ZCH):          # source chunk c2 = q2*8 + cr
                for cc in range(ZCH):      # dest chunk c = q*8 + cc
                    c2 = q2 * ZCH + cr
                    c = q * ZCH + cc
                    if c2 < c:
                        d2qb[cr, q * C + q2 * ZCH + cc] = (
                            Sendprev[c] - Send[c2] + logkap[c]
                        )
    m2 = np.exp(d2qb).astype(ml_dtypes.bfloat16)

    aux_all = np.concatenate([stackL, stackR],
                             axis=1).astype(ml_dtypes.bfloat16)

    # pack: awu = [Ublk bf16 | NEG tril mask bf16 | M2 on rows 0-7]
    m2pad = np.zeros((Q, NZQ * C), ml_dtypes.bfloat16)
    m2pad[0:ZCH] = m2
    awu = np.concatenate([ublk, _get_auxw(), m2pad], axis=1)
    return aux_all, awu


_AUXW = None


def _get_auxw():
    """bf16 [Q, GRP*Q]: tiled strict-upper NEG mask."""
    global _AUXW
    if _AUXW is None:
        j = np.arange(Q)[:, None]
        i = np.arange(Q)[None, :]
        one = np.where(i >= j, 0.0, NEG)
        _AUXW = np.tile(one, (1, GRP)).astype(ml_dtypes.bfloat16)
    return _AUXW


def _make_in_maps(z, pt):
    preps = [_host_prep(pt[b]) for b in range(B)]
    in_maps = []
    for core in range(N_CORES):
        b, dblk = divmod(core, ND)
        aux_all, awu = preps[b]
        z_slab = (
            z[b, :, dblk * DBLK:(dblk + 1) * DBLK]
            .reshape(C, Q, DBLK)
            .transpose(1, 0, 2)
            .reshape(Q, C * DBLK)
            .astype(ml_dtypes.bfloat16)
        )
        in_maps.append({
            # [awu | z] per row; one DMA delivers awu + z quarter 0
            "z_s": np.ascontiguousarray(
                np.concatenate([awu, z_slab], axis=1)),
            "aux_all": np.ascontiguousarray(aux_all),
        })
    return in_maps


def _unpack_out(res_core):
    """out_s [Q, C*DBLK] bf16 position-major -> [L, DBLK] f32."""
    return (
        res_core.astype(np.float32)
        .reshape(Q, C, DBLK)
        .transpose(1, 0, 2)
        .reshape(L, DBLK)
    )


def _install_ntff_shim():
    """Enable NTFF profiling under axon: shim the missing antenv.axon_hooks
    module and register the ctypes hook from trn_boot; skip the fileshare
    artifact upload (no bucket in this container)."""
    import sys
    import types
    import antenv

    if "antenv.axon_hooks" not in sys.modules:
        mod = types.ModuleType("antenv.axon_hooks")
        hook_box = [None]
        mod.set_axon_ntff_profile_hook = lambda h: hook_box.__setitem__(0, h)
        mod.get_axon_ntff_profile_hook = lambda: hook_box[0]
        mod._hook_box = hook_box
        sys.modules["antenv.axon_hooks"] = mod
        antenv.axon_hooks = mod
    mod = sys.modules["antenv.axon_hooks"]
    if mod.get_axon_ntff_profile_hook() is None:
        from trn_agent_boot.trn_boot import _ntff_profile_via_ctypes

        mod.set_axon_ntff_profile_hook(
            _ntff_profile_via_ctypes("/opt/axon/libaxon_pjrt.so")
        )
    import concourse.bass_utils as bu

    bu.upload_artifacts = lambda tmpdir: f"local://{tmpdir}"


def kernel(z, pt):
    global LAST_EXEC_NS
    from concourse.bass_utils import run_bass_kernel_spmd

    z = np.asarray(z, dtype=np.float32)
    pt = np.asarray(pt, dtype=np.float32)

    if "nc" not in _CTX:
        _CTX["nc"] = _build_program()
    nc = _CTX["nc"]

    in_maps = _make_in_maps(z, pt)

    trace = bool(int(os.environ.get("BASS_KERNEL_TRACE", "0")))
    if trace:
        try:
            _install_ntff_shim()
        except Exception:
            trace = False
    tmpdir = os.environ.get("BASS_KERNEL_TRACE_DIR") or None
    res = run_bass_kernel_spmd(
        nc, in_maps, list(range(N_CORES)), trace=trace, tmpdir=tmpdir
    )
    LAST_EXEC_NS = res.exec_time_ns

    out = np.empty((B, L, D), np.float32)
    for core in range(N_CORES):
        b, dblk = divmod(core, ND)
        out[b, :, dblk * DBLK:(dblk + 1) * DBLK] = _unpack_out(
            res.results[core]["out_s"]
        )
    return out

